# revision 3
# baseline (speedup 1.0000x reference)
"""LIF spiking-neuron recurrence on Trainium2 (8 NeuronCores), v3.

Reference semantics (TAU=1, THRESH=1, f32):
    mem = 0
    for t in range(T):
        mem = mem + x[t]
        spike[t] = (mem >= 1.0) ? 1.0 : 0.0
        mem = mem * (1 - spike[t])        # hard reset

Sharding: data-parallel over batch (B=128 -> 16 rows/core); per-core
shard viewed as [T, 128, 2048] and host-pre-transposed to [128, T, 2048].

v3 splits each step's 2048 columns into three zones so the adds leave
the DVE (the v1 bottleneck) and every engine lands under the measured
~3.4 us/step DMA floor (1 MiB x-load + 0.25 MiB int8 spike store at
~384 GB/s/core):
  zone A [0,A):    the load IS the add -- SWDGE dma accum_op=add onto the
                   membrane tile (inline CCE f32 add, probed exact; costs
                   2.26x SBUF-port bytes, so A is sized to the port slack)
  zone D [A,AD):   classic DVE tensor_tensor add (in-place, probed safe)
  zone P [AD,2048): PE adds: psum = I.T@x_t (+ I.T@mem), f32 identity
                   matmul probed bit-exact; x-matmul first so the
                   recurrence chain is only STT -> mem-matmul
Spike: ACT Sign(pre-1) -> int8 {-1,0,+1} in one op (ties at pre==1.0
give 0; host decodes spike = (s>=0); Sterbenz makes pre-1 exact near 1).
Reset: AD: one STT  nxt = (cur < 1) * cur  (ping-pong m0/m1);
       P (per stripe): STT  memp = (sgn_i8 < 0) * pre_psum  -- the int8
       sign tile is the non-PSUM operand, [s<0]*pre == hard reset, ties
       reset to 0 correctly.
Zone P runs as 2 column stripes (independent pipelines) to keep the
per-stripe dependency cycle under the step period; PSUM pre tiles are
double-buffered (4 banks total). D+P x-loads are one HWDGE transfer per
step (slice-level deps); stores ride the sync ring with them.
"""

import numpy as np

try:
    import concourse  # noqa: F401
except ImportError:  # pragma: no cover
    import sys

    for _p in ("/opt/trn_rl_repo", "/root/.axon_site/_ro/trn_rl_repo"):
        if _p not in sys.path:
            sys.path.insert(0, _p)

from concourse import bacc, mybir
from concourse.bass import MemorySpace
from concourse.bass_utils import run_bass_kernel_spmd
from concourse.mybir import ActivationFunctionType as AF
from concourse.mybir import AluOpType
from concourse.tile import TileContext

T, B, D = 64, 128, 16384
NCORES = 8
BL = B // NCORES  # 16 batch rows per core
P = 128  # SBUF partitions
F = (BL * D) // P  # 2048 free elements per timestep slab

ZA = 448          # accum-DMA zone cols
ZD = 640          # DVE-add zone cols
NPS = 2           # PE-zone stripes
AD = ZA + ZD
ZP = F - AD       # PE zone cols
PS = ZP // NPS    # cols per PE stripe
assert ZP % NPS == 0 and PS <= 512  # f32 moving-operand limit


def build_nc(t_steps=T, x_bufs=4, s_bufs=4):
    """Build + compile the per-core Bass program (identical on all cores)."""
    f32 = mybir.dt.float32
    i8 = mybir.dt.int8
    nc = bacc.Bacc(
        "TRN2", target_bir_lowering=False, debug=False, num_devices=NCORES
    )
    x_ext = nc.dram_tensor("x", [P, t_steps, F], f32, kind="ExternalInput")
    out_ext = nc.dram_tensor("out", [P, t_steps, F], i8, kind="ExternalOutput")
    eye_ext = nc.dram_tensor("eye", [P, P], f32, kind="ExternalInput")
    with TileContext(nc) as tc:
        with (
            tc.tile_pool(name="mp", bufs=1) as mp,
            tc.tile_pool(name="xp", bufs=x_bufs) as xp,
            tc.tile_pool(name="sp", bufs=s_bufs) as sp,
            tc.tile_pool(name="pp0", bufs=2, space=MemorySpace.PSUM) as pp0,
            tc.tile_pool(name="pp1", bufs=2, space=MemorySpace.PSUM) as pp1,
        ):
            pps = [pp0, pp1]
            eye = mp.tile([P, P], f32, name="eye")
            bm1 = mp.tile([P, 1], f32, name="bm1")
            m = [mp.tile([P, AD], f32, name=f"m{i}") for i in range(2)]
            memp = [mp.tile([P, PS], f32, name=f"memp{s}") for s in range(NPS)]
            nc.sync.dma_start(eye[:], eye_ext[:, :])
            nc.vector.memset(bm1[:], -1.0)
            for t in range(t_steps):
                cur = m[t % 2]
                nxt = m[(t + 1) % 2]
                sgn = sp.tile([P, F], i8, name="sgn")
                # zone A: accumulating load (the load IS the add)
                nc.gpsimd.dma_start(
                    cur[:, 0:ZA], x_ext[:, t, 0:ZA],
                    accum_op=AluOpType.bypass if t == 0 else AluOpType.add,
                )
                # zones D+P: one plain HWDGE load, consumed by slices
                xt = xp.tile([P, F - ZA], f32, name="xt")
                nc.sync.dma_start(xt[:], x_ext[:, t, ZA:F])
                # zone D add (in-place on the ping-pong tile)
                if t == 0:
                    nc.vector.tensor_copy(cur[:, ZA:AD], xt[:, 0:ZD])
                else:
                    nc.vector.tensor_tensor(
                        cur[:, ZA:AD], cur[:, ZA:AD], xt[:, 0:ZD], AluOpType.add
                    )
                # zone P adds: x-matmul first (prefetched), mem-matmul second
                pres = []
                for s in range(NPS):
                    pre = pps[s].tile([P, PS], f32, name=f"pre{s}")
                    pres.append(pre)
                    xs = xt[:, ZD + s * PS : ZD + (s + 1) * PS]
                    if t == 0:
                        nc.tensor.matmul(pre[:], eye[:], xs, start=True, stop=True)
                    else:
                        nc.tensor.matmul(pre[:], eye[:], xs, start=True, stop=False)
                        nc.tensor.matmul(
                            pre[:], eye[:], memp[s][:], start=False, stop=True
                        )
                # spikes: Sign(pre - 1) -> int8
                nc.scalar.activation(
                    sgn[:, 0:AD], cur[:], AF.Sign, bias=bm1[:], scale=1.0
                )
                for s in range(NPS):
                    nc.scalar.activation(
                        sgn[:, AD + s * PS : AD + (s + 1) * PS], pres[s][:],
                        AF.Sign, bias=bm1[:], scale=1.0,
                    )
                nc.sync.dma_start(out_ext[:, t, :], sgn[:])
                # resets
                if t < t_steps - 1:
                    nc.vector.scalar_tensor_tensor(
                        nxt[:], cur[:], 1.0, cur[:],
                        AluOpType.is_lt, AluOpType.mult,
                    )
                    for s in range(NPS):
                        nc.vector.scalar_tensor_tensor(
                            memp[s][:],
                            sgn[:, AD + s * PS : AD + (s + 1) * PS], 0.0,
                            pres[s][:], AluOpType.is_lt, AluOpType.mult,
                        )
    nc.compile()
    return nc


_cached_nc = None


def _get_nc():
    global _cached_nc
    if _cached_nc is None:
        _cached_nc = build_nc()
    return _cached_nc


def _shard(x):
    """Full [T, B, D] -> list of per-core [P, T, F] contiguous arrays."""
    eye = np.eye(P, dtype=np.float32)
    in_maps = []
    for c in range(NCORES):
        xc = x[:, c * BL : (c + 1) * BL, :].reshape(T, P, F).transpose(1, 0, 2)
        in_maps.append({"x": np.ascontiguousarray(xc), "eye": eye})
    return in_maps


def _gather(results):
    """Per-core [P, T, F] int8 sign outputs -> full [T, B, D] f32 spikes."""
    outs = [
        (np.asarray(results[c]["out"]) >= 0)
        .astype(np.float32)
        .transpose(1, 0, 2)
        .reshape(T, BL, D)
        for c in range(NCORES)
    ]
    return np.concatenate(outs, axis=1)


def run(x, trace=False, **kw):
    """Run on the 8 NeuronCores; returns (output, BassKernelResults)."""
    x = np.ascontiguousarray(np.asarray(x, dtype=np.float32))
    assert x.shape == (T, B, D), x.shape
    nc = _get_nc()
    res = run_bass_kernel_spmd(
        nc, _shard(x), core_ids=list(range(NCORES)), trace=trace, **kw
    )
    return _gather(res.results), res


def kernel(x: np.ndarray) -> np.ndarray:
    out, _ = run(x)
    return out
